# revision 8
# baseline (speedup 1.0000x reference)
"""AdaINResBlock1 (HiFi-GAN style) Trainium2 kernel, batch-parallel over 8 NeuronCores.

Layout: channels on partitions (4 groups x 128), time on the free axis.
Convs run as bf16 matmuls accumulating f32 in PSUM; instance-norm sums ride
the eviction accumulators; the style affine is fused into per-partition
scale/bias operands of ACT/DVE ops.

Schedule (v2):
- x streams in 16 quarter-transfers over the sync/scalar/vector HWDGE rings,
  emitted before anything else; per-chunk stats ride the arrival, split
  3-way (sums on DVE + gpsimd, squares on ACT + DVE) so no engine's queue
  serializes the stats tail.
- ALL conv weights load as bf16 gpsimd cast-DMAs (no staging tiles, no DVE
  casts); their squares for the weight-norm run on gpsimd mid-conv; the
  12 norm matmuls run on the PE right after each conv as barrier filler.
- conv1's weight-norm scale is NOT applied at eviction: the instance norm
  that follows is per-channel scale-invariant (g>0), so the scale folds
  into the stats chain as a per-channel epsilon EPS*||v||^2/g^2. W1's
  rsqrt chain disappears from the barrier critical path.
- conv matmuls run k-outer within each wave, so only the last third of a
  wave's matmuls depend on snake chunk tj+1 - the JIT snake needs 1.7us
  less lookahead at every wave boundary.
- at every stats barrier the PE is kept busy (HAM stays at K=8/8) by the
  norm matmuls plus a bridge of dummy matmuls: phase A fires at barrier
  entry, phase B gates on the first snake chunk of the next conv.
- snake: reassociated as (t + sin(wrap(t))^2 * sqS^2) * invA with
  t = sinS*x + sinB; Sin/Square on ACT, wrap/combine on DVE, angle on
  gpsimd. Every rsqrt runs on the DVE (bit-hack seed + Newton steps), so
  only the trig_and_small ACT table set is ever loaded.
- conv biases are algebraically absorbed by downstream instance norms;
  the accumulated conv2 bias is applied per chunk on gpsimd under the
  last conv2's windows; the last conv2 streams each chunk to DRAM.
"""

import math
import sys
from contextlib import ExitStack
from itertools import cycle

import numpy as np

try:
    import concourse.bass as bass
except ImportError:  # pragma: no cover
    sys.path.insert(0, "/opt/trn_rl_repo")
    import concourse.bass as bass

import concourse.tile as tile
from concourse import bacc, mybir

f32 = mybir.dt.float32
bf16 = mybir.dt.bfloat16
f16 = mybir.dt.float16
i32 = mybir.dt.int32
AF = mybir.ActivationFunctionType
OP = mybir.AluOpType
AX = mybir.AxisListType

B, T_FULL, C, S, KW = 8, 4096, 512, 64, 3
DILATIONS = (1, 3, 5)
EPS = 1e-5
G = C // 128          # 4 channel groups of 128 partitions
PADL = 5              # max dilation -> left/right zero pad for conv1 input
TCH = 512             # t-chunk width (one PSUM bank)
N_CORES = 8
QT = 4                # x arrival quarters


def build_nc(T=T_FULL, max_alpha=1.0, stop_after=None, n_iters=3):
    NT = T // TCH
    WAVES = [[t] for t in range(NT)]
    # ACT Sin is valid on [-pi, pi] only; each ADD_RANGE_WRAP pass unwraps one
    # period. Bound the angle by max_alpha * 9.
    N_WRAPS = max(1, int(math.ceil((max_alpha * 9.0 - math.pi) / (2 * math.pi))))
    PI = math.pi

    nc = bacc.Bacc()
    x_ext = nc.declare_dram_parameter("x", [C, T], f32, isOutput=False)
    s_ext = nc.declare_dram_parameter("s", [S, 1], f32, isOutput=False)
    fc1_w_ext = nc.declare_dram_parameter("fc1_w", [3, S, 2 * C], f32, isOutput=False)
    fc1_b_ext = nc.declare_dram_parameter("fc1_b", [3, 2 * C], f32, isOutput=False)
    alpha1_ext = nc.declare_dram_parameter("alpha1", [3, C], f32, isOutput=False)
    conv1_v_ext = nc.declare_dram_parameter("conv1_v", [3, KW, C, C], f32, isOutput=False)
    conv1_g_ext = nc.declare_dram_parameter("conv1_g", [3, C], f32, isOutput=False)
    conv1_b_ext = nc.declare_dram_parameter("conv1_b", [3, C], f32, isOutput=False)
    fc2_w_ext = nc.declare_dram_parameter("fc2_w", [3, S, 2 * C], f32, isOutput=False)
    fc2_b_ext = nc.declare_dram_parameter("fc2_b", [3, 2 * C], f32, isOutput=False)
    alpha2_ext = nc.declare_dram_parameter("alpha2", [3, C], f32, isOutput=False)
    conv2_v_ext = nc.declare_dram_parameter("conv2_v", [3, KW, C, C], f32, isOutput=False)
    conv2_g_ext = nc.declare_dram_parameter("conv2_g", [3, C], f32, isOutput=False)
    conv2_b_ext = nc.declare_dram_parameter("conv2_b", [3, C], f32, isOutput=False)
    out_ext = nc.declare_dram_parameter("out", [C, T], f32, isOutput=True)

    hw_rr = cycle([0, 1])  # sync / scalar HWDGE rings (late-phase small DMAs)

    with tile.TileContext(nc) as tc, ExitStack() as ctx:
        persist = ctx.enter_context(tc.tile_pool(name="persist", bufs=1))
        wpool = ctx.enter_context(tc.tile_pool(name="wpool", bufs=1))
        stage = ctx.enter_context(tc.tile_pool(name="stage", bufs=2))
        scr = ctx.enter_context(tc.tile_pool(name="scr", bufs=2))
        small = ctx.enter_context(tc.tile_pool(name="small", bufs=2))
        psc = ctx.enter_context(tc.tile_pool(name="psc", bufs=6, space="PSUM"))
        psm = ctx.enter_context(tc.tile_pool(name="psm", bufs=2, space="PSUM"))

        def hw_eng():
            return (nc.sync, nc.scalar)[next(hw_rr)]

        # ------------- persistent tiles -------------
        ones_col = persist.tile([128, 1], bf16, name="ones_col")
        ident1 = persist.tile([1, 1], f32, name="ident1")
        junk = persist.tile([128, 1], f32, name="junk")
        zero_col = persist.tile([128, 1], f32, name="zero_col")
        eps_col = persist.tile([128, 1], f32, name="eps_col")
        warm_src = persist.tile([128, TCH], bf16, name="warm_src")
        s_sb = persist.tile([S, 1], f32, name="s_sb")

        x_cur, b1pad, cb2pad = [], [], []
        for g in range(G):
            x_cur.append(persist.tile([128, T], f32, name=f"x_cur_{g}"))
            b1pad.append(persist.tile([128, PADL + T + PADL], bf16, name=f"b1pad_{g}"))
            cb2pad.append(persist.tile([128, 1 + T + 1], bf16, name=f"cb2pad_{g}"))

        # ---- phase 0: x quarter-DMAs first, on sync/scalar/vector rings ----
        TQ = T // QT
        x_engs = cycle([nc.sync, nc.scalar])
        for q in range(QT):
            for g in range(G):
                qsl = slice(q * TQ, (q + 1) * TQ)
                next(x_engs).dma_start(
                    out=x_cur[g][:, qsl], in_=x_ext[g * 128:(g + 1) * 128, qsl])

        # gpsimd: constants, pads, small vectors, conv weights (cast-DMAs)
        nc.gpsimd.memset(ones_col, 1.0)
        nc.gpsimd.memset(ident1, 1.0)
        nc.gpsimd.memset(zero_col, 0.0)
        nc.gpsimd.memset(eps_col, EPS)
        nc.gpsimd.memset(warm_src, 0.001)
        for g in range(G):
            nc.gpsimd.memset(b1pad[g][:, 0:PADL], 0.0)
            nc.gpsimd.memset(b1pad[g][:, PADL + T:PADL + T + PADL], 0.0)
            nc.gpsimd.memset(cb2pad[g][:, 0:1], 0.0)
            nc.gpsimd.memset(cb2pad[g][:, 1 + T:1 + T + 1], 0.0)
        # pin trig_and_small ACT table set (after the scalar ring's x issues)
        nc.scalar.activation(out=junk, in_=eps_col, func=AF.Sin, bias=zero_col)

        nc.gpsimd.dma_start(out=s_sb, in_=s_ext[:, :])

        fcw_tiles = {}

        def fc_dma(i, which, eng=None):
            wext = fc1_w_ext if which == 1 else fc2_w_ext
            fcw = stage.tile([S, 2 * C], f32, tag="fcw", name=f"fcw_{i}_{which}",
                             bufs=2)
            (eng or hw_eng()).dma_start(out=fcw, in_=wext[i])
            fcw_tiles[(i, which)] = fcw

        fc_dma(0, 1, eng=nc.gpsimd)
        fc_dma(0, 2, eng=nc.gpsimd)

        # batched per-channel vector loads: DRAM (3, C) -> (128, 3*G)
        def load_pcvec3(name, ext):
            t = persist.tile([128, 3 * G], f32, name=name)
            nc.gpsimd.dma_start(
                out=t, in_=ext.rearrange("i (g p) -> p (i g)", p=128))
            return t

        def lay(t, i):
            return t[:, i * G:(i + 1) * G]

        NCC = 2 * C // 128

        alpha_t = {1: load_pcvec3("alpha1_all", alpha1_ext),
                   2: load_pcvec3("alpha2_all", alpha2_ext)}
        g_t = {1: load_pcvec3("g1_all", conv1_g_ext),
               2: load_pcvec3("g2_all", conv2_g_ext)}
        cb_t = {1: load_pcvec3("cb1_all", conv1_b_ext),
                2: load_pcvec3("cb2_all", conv2_b_ext)}
        fcb_all = {}
        for which, bext in ((1, fc1_b_ext), (2, fc2_b_ext)):
            t = persist.tile([128, 3 * NCC], f32, name=f"fcb{which}_all")
            nc.gpsimd.dma_start(
                out=t, in_=bext.rearrange("i (c p) -> p (i c)", p=128))
            fcb_all[which] = t

        # ------------- weight prep: gpsimd cast-DMAs + gpsimd squares ------
        # k-major order matches the conv's k-outer matmul consumption order
        W_ORDER = [(k * G + ci, ci, k) for k in range(KW) for ci in range(G)]

        def wprep_state():
            return ([None] * (KW * G), [])

        def wprep_dma(i, which, state, lo, hi):
            vext = conv1_v_ext if which == 1 else conv2_v_ext
            W, _ = state
            for idx, ci, k in W_ORDER[lo:hi]:
                W[idx] = wpool.tile(
                    [128, TCH], bf16, tag=f"w{which}_{idx}",
                    name=f"w{which}_{i}_{idx}")
                nc.gpsimd.dma_start(
                    out=W[idx], in_=vext[i, k, ci * 128:(ci + 1) * 128, :])

        def wprep_sq(i, which, state, lo, hi):
            W, vsqs = state
            for idx, ci, k in W_ORDER[lo:hi]:
                vsq = scr.tile([128, TCH], bf16, tag="vsq", bufs=12,
                               name=f"vsq_{i}_{which}_{k}_{ci}")
                nc.gpsimd.tensor_tensor(out=vsq, in0=W[idx], in1=W[idx],
                                        op=OP.mult)
                vsqs.append(vsq)

        # W1_0 cast-DMAs at startup (squares injected into conv1_0's waves)
        st1 = wprep_state()
        wprep_dma(0, 1, st1, 0, KW * G)

        # ------------- input stats: quarter-wide ops as x arrives ----------
        # 1024-col ops halve the op count vs per-chunk; sums on DVE,
        # squares on ACT except the last quarter's (DVE) so the tail
        # finishes on two engines in parallel.
        xsum_cur = small.tile([128, G, QT], f32, tag="xsumi", name="xsum_in")
        xsq_cur = small.tile([128, G, QT], f32, tag="xsqi", name="xsq_in")
        for q in range(QT):
            for g in range(G):
                qsl = slice(q * TQ, (q + 1) * TQ)
                nc.vector.tensor_reduce(
                    xsum_cur[:, g, q:q + 1], x_cur[g][:, qsl],
                    axis=AX.X, op=OP.add)
                dst = scr.tile([128, TQ], bf16, tag="sqdw",
                               name=f"sqd_xin_{g}_{q}", bufs=3)
                if q < QT - 1:
                    nc.scalar.activation(
                        out=dst, in_=x_cur[g][:, qsl], func=AF.Square,
                        bias=zero_col, accum_out=xsq_cur[:, g, q:q + 1])
                else:
                    nc.vector.affine_mul_reduce(
                        out=dst, accum_out=xsq_cur[:, g, q:q + 1],
                        in0=x_cur[g][:, qsl], in1=x_cur[g][:, qsl],
                        scale=1.0, bias=0.0)

        # ------------- DVE rsqrt (no ACT sqrt -> no table switch) ----------
        def emit_rsqrt(dst, v, tag, iters=2):
            nc.vector.tensor_scalar(
                dst.bitcast(i32), v.bitcast(i32), 1, None,
                OP.logical_shift_right)
            nc.vector.tensor_scalar(
                dst.bitcast(i32), dst.bitcast(i32), -1, 0x5F3759DF,
                OP.mult, OP.add)
            t = small.tile(list(v.shape), f32, tag=f"nr_{tag}", name=f"nr_{tag}")
            for _ in range(iters):
                nc.vector.tensor_tensor(out=t, in0=dst, in1=dst, op=OP.mult)
                nc.vector.tensor_tensor(out=t, in0=t, in1=v, op=OP.mult)
                nc.vector.tensor_scalar(t, t, -0.5, 1.5, OP.mult, OP.add)
                nc.vector.tensor_tensor(out=dst, in0=dst, in1=t, op=OP.mult)
            return dst

        # 1/sqrt(alpha), 1/alpha, 1/g^2 for all layers (off the critical path)
        sqS_t, invA_t, rg2_t = {}, {}, {}
        for which in (1, 2):
            allt = persist.tile([128, 3 * G], f32, name=f"sqS{which}_all")
            emit_rsqrt(allt, alpha_t[which][:, 0:3 * G], f"sa{which}")
            sqS_t[which] = allt
            inv = persist.tile([128, 3 * G], f32, name=f"invA{which}_all")
            nc.vector.reciprocal(inv, alpha_t[which][:, 0:3 * G])
            invA_t[which] = inv
        rg = persist.tile([128, 3 * G], f32, name="rg1")
        nc.vector.reciprocal(rg, g_t[1][:, 0:3 * G])
        rg2 = persist.tile([128, 3 * G], f32, name="rg2_1")
        nc.vector.tensor_tensor(out=rg2, in0=rg, in1=rg, op=OP.mult)
        rg2_t[1] = rg2

        # ------------- fc + style coefficients -------------
        def fc_mm(i, which):
            fcw = fcw_tiles.pop((i, which))
            hps = psm.tile([128, NCC], f32, tag="pm", name=f"hps_{i}_{which}")
            for cc in range(NCC):
                nc.tensor.matmul(
                    hps[:, cc:cc + 1],
                    fcw[:, cc * 128:(cc + 1) * 128],
                    s_sb,
                    start=(cc == 0), stop=(cc == NCC - 1))
            h_sb = small.tile([128, NCC], f32, tag="hsb",
                              name=f"h_{i}_{which}", bufs=2)
            nc.vector.tensor_tensor(
                out=h_sb, in0=hps,
                in1=fcb_all[which][:, i * NCC:(i + 1) * NCC], op=OP.add)
            return h_sb

        def emit_coef(tag, h_sb, alpha):
            q = small.tile([128, G], f32, tag="q", name=f"q_{tag}")
            nc.vector.scalar_tensor_tensor(
                out=q, in0=h_sb[:, 0:G], scalar=1.0, in1=alpha,
                op0=OP.add, op1=OP.mult)
            ab = small.tile([128, G], f32, tag="ab", name=f"ab_{tag}")
            nc.vector.tensor_tensor(out=ab, in0=h_sb[:, G:2 * G], in1=alpha,
                                    op=OP.mult)
            return q, ab

        hcoef = {}
        for which in (1, 2):
            h = fc_mm(0, which)
            hcoef[(0, which)] = emit_coef(f"l0_{which}", h,
                                          lay(alpha_t[which], 0))

        # ------------- weight-norm, split PE / DVE phases -------------
        def wprep_norm_mm(i, which, state):
            """PE norm matmuls + transposes; ends with nsq in SBUF (via one
            DVE copy so the psm banks recycle fast). Barrier PE filler."""
            W, vsqs = state
            normps = psm.tile([1, C], f32, tag="pm", name=f"norm_{i}_{which}")
            for n, vsq in enumerate(vsqs):
                nc.tensor.matmul(
                    normps, ones_col, vsq,
                    start=(n == 0), stop=(n == KW * G - 1))
            nrow = small.tile([1, C], f32, tag="nrow", name=f"nrow_{i}_{which}",
                              bufs=2)
            nc.scalar.activation(out=nrow, in_=normps, func=AF.Copy)
            ps_t = psm.tile([128, G], f32, tag="pm", name=f"wnt_{i}_{which}")
            for g in range(G):
                nc.tensor.matmul(
                    ps_t[:, g:g + 1], nrow[0:1, g * 128:(g + 1) * 128], ident1,
                    is_transpose=True, start=(g == 0), stop=(g == G - 1))
            nsq = small.tile([128, G], f32, tag=f"nsq{which}",
                             name=f"nsq_{i}_{which}", bufs=2)
            nc.vector.tensor_scalar(nsq, ps_t, 0.0, None, OP.add)
            return W, nsq

        def wprep_epsc(i, nsq):
            """Per-channel epsilon EPS*||v||^2/g^2 for the wnS1-free stats."""
            epsc = small.tile([128, G], f32, tag="epsc", name=f"epsc_{i}",
                              bufs=2)
            nc.vector.scalar_tensor_tensor(
                out=epsc, in0=nsq, scalar=EPS, in1=lay(rg2_t[1], i),
                op0=OP.mult, op1=OP.mult)
            return epsc

        def wprep_fin(i, which, nsq):
            """g/||v|| via DVE rsqrt (conv2 only: its scale must be applied
            at eviction since the residual add isn't normalized)."""
            wnS = small.tile([128, G], f32, tag=f"wns{which}",
                             name=f"wns_{i}_{which}", bufs=2)
            emit_rsqrt(wnS, nsq, f"wn{which}")
            nc.vector.tensor_tensor(out=wnS, in0=wnS, in1=lay(g_t[which], i),
                                    op=OP.mult)
            return wnS

        # ------------- warm-bridge dummy matmuls -------------
        def warm_fill(tag, n, rhs=None):
            """n back-to-back matmuls into a scratch psm tile: keeps the PE
            HAM at K=8/8 through a stats barrier. rhs gates the start."""
            if n <= 0:
                return
            mv = rhs if rhs is not None else warm_src
            st = eps_col if mv.dtype == f32 else ones_col
            jp = psm.tile([1, TCH], f32, tag="pm", name=f"jk_{tag}")
            for m in range(n):
                nc.tensor.matmul(jp, st, mv[:, 0:TCH] if mv.shape[1] > TCH
                                 else mv, start=(m == 0), stop=(m == n - 1))

        # ------------- stats chain -------------
        def emit_stats(tag, sum3d, sq3d, coef, epsc=None):
            """sinS/sinB from per-chunk raw sums. epsc: per-channel epsilon
            tensor (wnS1-folded path) or None (EPS scalar, exact scale)."""
            q, ab = coef
            sums = small.tile([128, G], f32, tag="sums", name=f"sums_{tag}")
            nc.vector.tensor_reduce(sums, sum3d, axis=AX.X, op=OP.add)
            sqs = small.tile([128, G], f32, tag="sqs", name=f"sqs_{tag}")
            nc.vector.tensor_reduce(sqs, sq3d, axis=AX.X, op=OP.add)
            mu = small.tile([128, G], f32, tag="mu", name=f"mu_{tag}")
            nc.vector.tensor_scalar(mu, sums, 1.0 / T, None, OP.mult)
            if epsc is None:
                nc.vector.tensor_scalar(sqs, sqs, 1.0 / T, EPS, OP.mult, OP.add)
            else:
                nc.vector.tensor_scalar(sqs, sqs, 1.0 / T, None, OP.mult)
            var = small.tile([128, G], f32, tag="var", name=f"var_{tag}")
            nc.vector.tensor_tensor(out=var, in0=mu, in1=mu, op=OP.mult)
            nc.vector.tensor_tensor(out=var, in0=sqs, in1=var, op=OP.subtract)
            if epsc is not None:
                nc.vector.tensor_tensor(out=var, in0=var, in1=epsc, op=OP.add)
            istd = small.tile([128, G], f32, tag="istd", name=f"istd_{tag}")
            emit_rsqrt(istd, var, "istd", iters=1)
            sinS = small.tile([128, G], f32, tag="sinS", name=f"sinS_{tag}")
            nc.vector.tensor_tensor(out=sinS, in0=q, in1=istd, op=OP.mult)
            sinB = small.tile([128, G], f32, tag="sinB", name=f"sinB_{tag}")
            nc.vector.tensor_tensor(out=sinB, in0=mu, in1=sinS, op=OP.mult)
            nc.vector.tensor_tensor(out=sinB, in0=ab, in1=sinB, op=OP.subtract)
            return sinS, sinB

        def snake_chunk(tag, cj, src_fn, dst_fn, sinS, sinB, sqS, invA):
            """dst = (t + sin(wrap(t))^2 * sqS^2) * invA, t = sinS*x + sinB,
            for one 512-col chunk, all groups."""
            csl = slice(cj * TCH, (cj + 1) * TCH)
            for g in range(G):
                t_g = scr.tile([128, TCH], f16, tag="ang",
                               name=f"ang_{tag}_{cj}_{g}", bufs=5)
                w_g = scr.tile([128, TCH], f16, tag="wrap",
                               name=f"wrap_{tag}_{cj}_{g}", bufs=5)
                sin_g = scr.tile([128, TCH], f16, tag="sin",
                                 name=f"sin_{tag}_{cj}_{g}", bufs=5)
                nc.gpsimd.tensor_scalar(
                    t_g, src_fn(g)[:, csl],
                    sinS[:, g:g + 1], sinB[:, g:g + 1],
                    OP.mult, OP.add)
                nc.vector.add_range_wrap(w_g, t_g, 0.0, PI, 2.0 * PI)
                for _ in range(N_WRAPS - 1):
                    nc.vector.add_range_wrap(w_g, w_g, 0.0, PI, 2.0 * PI)
                nc.scalar.activation(out=sin_g, in_=w_g,
                                     func=AF.Sin, bias=zero_col)
                nc.scalar.activation(out=sin_g, in_=sin_g,
                                     func=AF.Square,
                                     scale=sqS[:, g:g + 1], bias=zero_col)
                nc.vector.scalar_tensor_tensor(
                    out=dst_fn(g)[:, csl], in0=t_g,
                    scalar=invA[:, g:g + 1], in1=sin_g,
                    op0=OP.mult, op1=OP.add)

        def emit_sq_chunk(src_ap, slot_ap, parity, tag):
            dst = scr.tile([128, TCH], bf16, tag="sqd", name=f"sqd_{tag}",
                           bufs=3)
            if parity:
                nc.scalar.activation(out=dst, in_=src_ap, func=AF.Square,
                                     bias=zero_col, accum_out=slot_ap)
            else:
                nc.vector.affine_mul_reduce(
                    out=dst, accum_out=slot_ap, in0=src_ap, in1=src_ap,
                    scale=1.0, bias=0.0)

        def emit_conv(tag, W, src_pad, pad, d, evict_fn, post_fn=None,
                      snake_fn=None, inject=None, pre_fn=None):
            """Conv waves, k-outer (only the last k-group of matmuls depends
            on snake chunk tj+1), with JIT snake production and mid-conv
            injection hooks."""
            produced = 0
            for wi, wave in enumerate(WAVES):
                if snake_fn is not None:
                    need = min(NT, wave[-1] + 2)
                    while produced < need:
                        snake_fn(produced)
                        produced += 1
                if pre_fn is not None:
                    pre_fn(wi, wave)
                for co in range(G):
                    pts = [
                        psc.tile([128, TCH], f32, tag="pc",
                                 name=f"ps_{tag}_{co}_{tj}")
                        for tj in wave
                    ]
                    for k in range(KW):
                        for ci in range(G):
                            first = (k == 0 and ci == 0)
                            last = (k == KW - 1 and ci == G - 1)
                            for pt, tj in zip(pts, wave):
                                off = pad + tj * TCH + (k - 1) * d
                                nc.tensor.matmul(
                                    pt,
                                    W[k * G + ci][:, co * 128:(co + 1) * 128],
                                    src_pad[ci][:, off:off + TCH],
                                    start=first, stop=last)
                    for pt, tj in zip(pts, wave):
                        evict_fn(co, tj, pt)
                        if post_fn is not None:
                            post_fn(co, tj, pt)
                if inject is not None and wi in inject:
                    for fn in inject.pop(wi):
                        fn()
            if snake_fn is not None:
                while produced < NT:
                    snake_fn(produced)
                    produced += 1

        # stats for the input of conv1_0 (exact scale -> scalar EPS path)
        sinS1, sinB1 = emit_stats("a1_0", xsum_cur, xsq_cur, hcoef.pop((0, 1)))
        # PE warm-up bridge: gated on the last x quarter's arrival
        warm_fill("st", 8, rhs=x_cur[G - 1][:, T - TCH:T])

        # ------------- iterations -------------
        pending_bias = None
        st2 = None

        for i in range(n_iters):
            d = DILATIONS[i]
            coef2 = hcoef.pop((i, 2))

            def snake1(cj, i=i, sS=sinS1, sB=sinB1):
                snake_chunk(f"s1_{i}", cj,
                            src_fn=lambda g: x_cur[g][:, 0:T],
                            dst_fn=lambda g: b1pad[g][:, PADL:PADL + T],
                            sinS=sS, sinB=sB,
                            sqS=lay(sqS_t[1], i), invA=lay(invA_t[1], i))

            c1sum = small.tile([128, G, NT], f32, tag="c1sum", name=f"c1sum_{i}")
            c1sq = small.tile([128, G, NT], f32, tag="c1sq", name=f"c1sq_{i}")

            # conv1 evicts RAW output: the weight-norm scale folds into the
            # following instance norm (per-channel epsilon), and conv1's bias
            # is absorbed exactly by its mean subtraction.
            def evict1(co, tj, pt, c1sum=c1sum):
                dst = cb2pad[co][:, 1 + tj * TCH: 1 + (tj + 1) * TCH]
                nc.scalar.activation(
                    out=dst, in_=pt, func=AF.Identity,
                    bias=zero_col, accum_out=c1sum[:, co, tj:tj + 1])

            def post1(co, tj, pt, i=i, c1sq=c1sq):
                src_ap = cb2pad[co][:, 1 + tj * TCH: 1 + (tj + 1) * TCH]
                dst = scr.tile([128, TCH], bf16, tag="sqd",
                               name=f"sqd_c1_{i}_{co}_{tj}", bufs=3)
                nc.vector.affine_mul_reduce(
                    out=dst, accum_out=c1sq[:, co, tj:tj + 1],
                    in0=src_ap, in1=src_ap, scale=1.0, bias=0.0)

            # conv2_i weight pipeline rides conv1_i's waves
            st2 = wprep_state()
            nsq1_hold = {}
            if i == 0:
                # W1_0's squares ride conv1_0's first waves; its norm matmuls
                # run mid-conv (wave 4) so the 12 vsq buffers recycle before
                # W2_0's squares need them (deadlock otherwise: the gpsimd
                # queue would stall ahead of the JIT snake angles).
                inject1 = {
                    0: [lambda: wprep_sq(0, 1, st1, 0, 6)],
                    1: [lambda: wprep_sq(0, 1, st1, 6, 12)],
                    2: [lambda st2=st2: wprep_dma(0, 2, st2, 0, 6)],
                    3: [lambda st2=st2: wprep_dma(0, 2, st2, 6, 12)],
                    4: [lambda: nsq1_hold.update(
                        n=wprep_norm_mm(0, 1, st1)[1])],
                    5: [lambda st2=st2: wprep_sq(0, 2, st2, 0, 6)],
                    6: [lambda st2=st2: wprep_sq(0, 2, st2, 6, 12)],
                }
            else:
                inject1 = {
                    2: [lambda st2=st2, i=i: wprep_dma(i, 2, st2, 0, 6)],
                    3: [lambda st2=st2, i=i: wprep_dma(i, 2, st2, 6, 12)],
                    4: [lambda st2=st2, i=i: wprep_sq(i, 2, st2, 0, 6)],
                    5: [lambda st2=st2, i=i: wprep_sq(i, 2, st2, 6, 12)],
                }
            if i < n_iters - 1:
                inject1.setdefault(6, []).append(
                    lambda i=i: (fc_dma(i + 1, 1), fc_dma(i + 1, 2)))

            def pre1(wi, wave, i=i):
                if wi == 0:
                    # phase-B warm bridge: gated on snake1 chunk 0, group 0
                    warm_fill(f"b1_{i}", 12, rhs=b1pad[0][:, PADL:PADL + TCH])

            emit_conv(f"c1_{i}", st1[0], b1pad, PADL, d, evict1, post1,
                      snake_fn=snake1, inject=inject1, pre_fn=pre1)

            # barrier c1_i -> c2_i: norm matmuls as PE filler, then bridge
            _, nsq2 = wprep_norm_mm(i, 2, st2)
            warm_fill(f"a2_{i}", 10)
            if i == 0:
                epsc1 = wprep_epsc(0, nsq1_hold["n"])
            sinS2, sinB2 = emit_stats(f"a2_{i}", c1sum, c1sq, coef2,
                                      epsc=epsc1)
            wnS2 = wprep_fin(i, 2, nsq2)

            def snake2(cj, i=i, sS=sinS2, sB=sinB2):
                snake_chunk(f"s2_{i}", cj,
                            src_fn=lambda g: cb2pad[g][:, 1:1 + T],
                            dst_fn=lambda g: cb2pad[g][:, 1:1 + T],
                            sinS=sS, sinB=sB,
                            sqS=lay(sqS_t[2], i), invA=lay(invA_t[2], i))

            # conv2 bias: accumulate; apply per chunk under the last conv2
            if pending_bias is None:
                pending_bias = small.tile([128, G], f32, tag="pend",
                                          name="pending_bias", bufs=1)
                nc.vector.tensor_copy(pending_bias, lay(cb_t[2], i))
            else:
                nc.vector.tensor_tensor(out=pending_bias, in0=pending_bias,
                                        in1=lay(cb_t[2], i), op=OP.add)

            last = (i == n_iters - 1)

            xsum_nxt = small.tile([128, G, NT], f32, tag="xsum", name=f"xsum_{i}")
            xsq_nxt = small.tile([128, G, NT], f32, tag="xsq", name=f"xsq_{i}")

            def evict2(co, tj, pt, wnS2=wnS2, xsum_nxt=xsum_nxt):
                sl = x_cur[co][:, tj * TCH:(tj + 1) * TCH]
                nc.vector.scalar_tensor_tensor(
                    out=sl, in0=pt, scalar=wnS2[:, co:co + 1], in1=sl,
                    op0=OP.mult, op1=OP.add,
                    accum_out=xsum_nxt[:, co, tj:tj + 1])

            def post2(co, tj, pt, i=i, xsq_nxt=xsq_nxt, last=last):
                sl = x_cur[co][:, tj * TCH:(tj + 1) * TCH]
                if last:
                    hw_eng().dma_start(
                        out=out_ext[co * 128:(co + 1) * 128,
                                    tj * TCH:(tj + 1) * TCH],
                        in_=sl)
                else:
                    emit_sq_chunk(sl, xsq_nxt[:, co, tj:tj + 1],
                                  parity=1, tag=f"x_{i}_{co}_{tj}")

            # next layer's conv1 weights + fc ride conv2_i's waves
            inject2 = {}
            st1n = wprep_state()
            if i < n_iters - 1:
                def fc_next(i=i):
                    for which in (1, 2):
                        h = fc_mm(i + 1, which)
                        hcoef[(i + 1, which)] = emit_coef(
                            f"l{i + 1}_{which}", h,
                            lay(alpha_t[which], i + 1))
                inject2 = {
                    2: [lambda st1n=st1n, i=i: wprep_dma(i + 1, 1, st1n, 0, 6)],
                    3: [lambda st1n=st1n, i=i: wprep_dma(i + 1, 1, st1n, 6, 12)],
                    4: [lambda st1n=st1n, i=i: wprep_sq(i + 1, 1, st1n, 0, 6)],
                    5: [lambda st1n=st1n, i=i: wprep_sq(i + 1, 1, st1n, 6, 12)],
                    6: [fc_next],
                }

            def pre2(wi, wave, i=i, last=last, pending_bias=pending_bias):
                if wi == 0:
                    warm_fill(f"b2_{i}", 12, rhs=cb2pad[0][:, 1:1 + TCH])
                if last:
                    # deferred-bias adds on gpsimd (DVE is saturated here)
                    for co in range(G):
                        for tj in wave:
                            sl = x_cur[co][:, tj * TCH:(tj + 1) * TCH]
                            nc.gpsimd.tensor_scalar(
                                sl, sl, pending_bias[:, co:co + 1], None,
                                OP.add)

            emit_conv(f"c2_{i}", st2[0], cb2pad, 1, 1, evict2, post2,
                      snake_fn=snake2, inject=inject2, pre_fn=pre2)
            xsum_cur, xsq_cur = xsum_nxt, xsq_nxt

            if i < n_iters - 1:
                # barrier c2_i -> c1_{i+1}
                _, nsq1n = wprep_norm_mm(i + 1, 1, st1n)
                warm_fill(f"a1_{i + 1}", 10)
                epsc1 = wprep_epsc(i + 1, nsq1n)
                sinS1, sinB1 = emit_stats(
                    f"a1_{i + 1}", xsum_cur, xsq_cur, hcoef.pop((i + 1, 1)))
                st1 = st1n

    return nc


def make_in_maps(inputs, T=T_FULL):
    npf = lambda v: np.asarray(v, dtype=np.float32)
    x = npf(inputs["x"])
    s = npf(inputs["s"])
    shared = {
        "fc1_w": npf(inputs["fc1_w"]),
        "fc1_b": npf(inputs["fc1_b"]),
        "alpha1": npf(inputs["alpha1"]).reshape(3, C),
        "conv1_v": npf(inputs["conv1_v"]),
        "conv1_g": npf(inputs["conv1_g"]),
        "conv1_b": npf(inputs["conv1_b"]),
        "fc2_w": npf(inputs["fc2_w"]),
        "fc2_b": npf(inputs["fc2_b"]),
        "alpha2": npf(inputs["alpha2"]).reshape(3, C),
        "conv2_v": npf(inputs["conv2_v"]),
        "conv2_g": npf(inputs["conv2_g"]),
        "conv2_b": npf(inputs["conv2_b"]),
    }
    in_maps = []
    for b in range(N_CORES):
        m = dict(shared)
        m["x"] = np.ascontiguousarray(x[b, :T, :].T)
        m["s"] = np.ascontiguousarray(s[b].reshape(S, 1))
        in_maps.append(m)
    return in_maps


_CACHED = {}


def kernel(**inputs) -> np.ndarray:
    from concourse.bass_utils import run_bass_kernel_spmd

    max_alpha = float(max(np.abs(np.asarray(inputs["alpha1"])).max(),
                          np.abs(np.asarray(inputs["alpha2"])).max()))
    key = ("nc", max_alpha)
    if key not in _CACHED:
        nc = build_nc(T_FULL, max_alpha=max_alpha)
        nc.finalize()
        _CACHED[key] = nc
    nc = _CACHED[key]
    in_maps = make_in_maps(inputs, T_FULL)
    res = run_bass_kernel_spmd(nc, in_maps, core_ids=list(range(N_CORES)))
    out = np.stack(
        [np.asarray(res.results[i]["out"]).T for i in range(N_CORES)], axis=0)
    return np.ascontiguousarray(out).astype(np.float32)


# revision 19
# speedup vs baseline: 1.3230x; 1.3230x over previous
"""AdaINResBlock1 (HiFi-GAN style) Trainium2 kernel, batch-parallel over 8 NeuronCores.

Layout: channels on partitions (4 groups x 128), time on the free axis.
Convs run as bf16 matmuls accumulating f32 in PSUM; instance-norm sums ride
the eviction accumulators; the style affine is fused into per-partition
scale/bias operands of ACT/DVE ops.

Schedule (v2):
- x streams in 16 quarter-transfers over the sync/scalar/vector HWDGE rings,
  emitted before anything else; per-chunk stats ride the arrival, split
  3-way (sums on DVE + gpsimd, squares on ACT + DVE) so no engine's queue
  serializes the stats tail.
- ALL conv weights load as bf16 gpsimd cast-DMAs (no staging tiles, no DVE
  casts); their squares for the weight-norm run on gpsimd mid-conv; the
  12 norm matmuls run on the PE right after each conv as barrier filler.
- conv1's weight-norm scale is NOT applied at eviction: the instance norm
  that follows is per-channel scale-invariant (g>0), so the scale folds
  into the stats chain as a per-channel epsilon EPS*||v||^2/g^2. W1's
  rsqrt chain disappears from the barrier critical path.
- conv matmuls run k-outer within each wave, so only the last third of a
  wave's matmuls depend on snake chunk tj+1 - the JIT snake needs 1.7us
  less lookahead at every wave boundary.
- at every stats barrier the PE is kept busy (HAM stays at K=8/8) by the
  norm matmuls plus a bridge of dummy matmuls: phase A fires at barrier
  entry, phase B gates on the first snake chunk of the next conv.
- snake: reassociated as (t + sin(wrap(t))^2 * sqS^2) * invA with
  t = sinS*x + sinB; Sin/Square on ACT, wrap/combine on DVE, angle on
  gpsimd. Every rsqrt runs on the DVE (bit-hack seed + Newton steps), so
  only the trig_and_small ACT table set is ever loaded.
- conv biases are algebraically absorbed by downstream instance norms;
  the accumulated conv2 bias is applied per chunk on gpsimd under the
  last conv2's windows; the last conv2 streams each chunk to DRAM.
"""

import math
import sys
from contextlib import ExitStack
from itertools import cycle

import numpy as np

try:
    import concourse.bass as bass
except ImportError:  # pragma: no cover
    sys.path.insert(0, "/opt/trn_rl_repo")
    import concourse.bass as bass

import concourse.tile as tile
from concourse import bacc, mybir

f32 = mybir.dt.float32
bf16 = mybir.dt.bfloat16
f16 = mybir.dt.float16
i32 = mybir.dt.int32
AF = mybir.ActivationFunctionType
OP = mybir.AluOpType
AX = mybir.AxisListType

B, T_FULL, C, S, KW = 8, 4096, 512, 64, 3
DILATIONS = (1, 3, 5)
EPS = 1e-5
G = C // 128          # 4 channel groups of 128 partitions
PADL = 5              # max dilation -> left/right zero pad for conv1 input
TCH = 512             # t-chunk width (one PSUM bank)
N_CORES = 8
QT = 4                # x arrival quarters


def build_nc(T=T_FULL, max_alpha=1.0, stop_after=None, n_iters=3):
    NT = T // TCH
    WAVES = [[t] for t in range(NT)]
    # ACT Sin is valid on [-pi, pi] only; each ADD_RANGE_WRAP pass unwraps one
    # period. Bound the angle by max_alpha * 9.
    N_WRAPS = max(1, int(math.ceil((max_alpha * 9.0 - math.pi) / (2 * math.pi))))
    PI = math.pi

    nc = bacc.Bacc()
    x_ext = nc.declare_dram_parameter("x", [C, T], f32, isOutput=False)
    s_ext = nc.declare_dram_parameter("s", [S, 1], f32, isOutput=False)
    fc1_w_ext = nc.declare_dram_parameter("fc1_w", [3, S, 2 * C], f32, isOutput=False)
    fc1_b_ext = nc.declare_dram_parameter("fc1_b", [3, 2 * C], f32, isOutput=False)
    alpha1_ext = nc.declare_dram_parameter("alpha1", [3, C], f32, isOutput=False)
    # conv weights ship host-cast to bf16: plain HWDGE loads, half the bytes
    conv1_v_ext = nc.declare_dram_parameter("conv1_v", [3, KW, C, C], bf16, isOutput=False)
    conv1_g_ext = nc.declare_dram_parameter("conv1_g", [3, C], f32, isOutput=False)
    conv1_b_ext = nc.declare_dram_parameter("conv1_b", [3, C], f32, isOutput=False)
    fc2_w_ext = nc.declare_dram_parameter("fc2_w", [3, S, 2 * C], f32, isOutput=False)
    fc2_b_ext = nc.declare_dram_parameter("fc2_b", [3, 2 * C], f32, isOutput=False)
    alpha2_ext = nc.declare_dram_parameter("alpha2", [3, C], f32, isOutput=False)
    conv2_v_ext = nc.declare_dram_parameter("conv2_v", [3, KW, C, C], bf16, isOutput=False)
    conv2_g_ext = nc.declare_dram_parameter("conv2_g", [3, C], f32, isOutput=False)
    conv2_b_ext = nc.declare_dram_parameter("conv2_b", [3, C], f32, isOutput=False)
    out_ext = nc.declare_dram_parameter("out", [C, T], f32, isOutput=True)

    hw_rr = cycle([0, 1])  # sync / scalar HWDGE rings (late-phase small DMAs)

    with tile.TileContext(nc) as tc, ExitStack() as ctx:
        persist = ctx.enter_context(tc.tile_pool(name="persist", bufs=1))
        wpool = ctx.enter_context(tc.tile_pool(name="wpool", bufs=1))
        stage = ctx.enter_context(tc.tile_pool(name="stage", bufs=2))
        scr = ctx.enter_context(tc.tile_pool(name="scr", bufs=2))
        small = ctx.enter_context(tc.tile_pool(name="small", bufs=2))
        psc = ctx.enter_context(tc.tile_pool(name="psc", bufs=6, space="PSUM"))
        psm = ctx.enter_context(tc.tile_pool(name="psm", bufs=2, space="PSUM"))

        def hw_eng():
            return (nc.sync, nc.scalar)[next(hw_rr)]

        # ------------- persistent tiles -------------
        ones_col = persist.tile([128, 1], bf16, name="ones_col")
        ident1 = persist.tile([1, 1], f32, name="ident1")
        junk = persist.tile([128, 1], f32, name="junk")
        zero_col = persist.tile([128, 1], f32, name="zero_col")
        eps_col = persist.tile([128, 1], f32, name="eps_col")
        warm_src = persist.tile([128, TCH], bf16, name="warm_src")
        s_sb = persist.tile([S, 1], f32, name="s_sb")

        x_cur, b1pad, cb2pad = [], [], []
        for g in range(G):
            x_cur.append(persist.tile([128, T], f32, name=f"x_cur_{g}"))
            b1pad.append(persist.tile([128, PADL + T + PADL], bf16, name=f"b1pad_{g}"))
            cb2pad.append(persist.tile([128, 1 + T + 1], bf16, name=f"cb2pad_{g}"))

        # a few constants before the gpsimd ring's x issues (cheap memsets)
        nc.gpsimd.memset(ones_col, 1.0)
        nc.gpsimd.memset(ident1, 1.0)
        nc.gpsimd.memset(zero_col, 0.0)
        nc.gpsimd.memset(eps_col, EPS)
        nc.gpsimd.memset(warm_src, 0.001)

        # ---- phase 0: x quarter-DMAs first, spread over all three rings ----
        TQ = T // QT
        x_engs = cycle([nc.scalar, nc.gpsimd, nc.sync])
        for q in range(QT):
            for g in range(G):
                qsl = slice(q * TQ, (q + 1) * TQ)
                next(x_engs).dma_start(
                    out=x_cur[g][:, qsl], in_=x_ext[g * 128:(g + 1) * 128, qsl])

        for g in range(G):
            nc.gpsimd.memset(b1pad[g][:, 0:PADL], 0.0)
            nc.gpsimd.memset(b1pad[g][:, PADL + T:PADL + T + PADL], 0.0)
            nc.gpsimd.memset(cb2pad[g][:, 0:1], 0.0)
            nc.gpsimd.memset(cb2pad[g][:, 1 + T:1 + T + 1], 0.0)
        # pin trig_and_small ACT table set (after the scalar ring's x issues)
        nc.scalar.activation(out=junk, in_=eps_col, func=AF.Sin, bias=zero_col)

        nc.gpsimd.dma_start(out=s_sb, in_=s_ext[:, :])

        fcw_tiles = {}

        def fc_dma(i, which, eng=None):
            wext = fc1_w_ext if which == 1 else fc2_w_ext
            fcw = stage.tile([S, 2 * C], f32, tag="fcw", name=f"fcw_{i}_{which}",
                             bufs=2)
            (eng or nc.sync).dma_start(out=fcw, in_=wext[i])
            fcw_tiles[(i, which)] = fcw

        fc_dma(0, 1)
        fc_dma(0, 2)

        # batched per-channel vector loads: DRAM (3, C) -> (128, 3*G)
        def load_pcvec3(name, ext):
            t = persist.tile([128, 3 * G], f32, name=name)
            nc.gpsimd.dma_start(
                out=t, in_=ext.rearrange("i (g p) -> p (i g)", p=128))
            return t

        def lay(t, i):
            return t[:, i * G:(i + 1) * G]

        NCC = 2 * C // 128

        alpha_t = {1: load_pcvec3("alpha1_all", alpha1_ext),
                   2: load_pcvec3("alpha2_all", alpha2_ext)}
        g_t = {1: load_pcvec3("g1_all", conv1_g_ext),
               2: load_pcvec3("g2_all", conv2_g_ext)}
        cb_t = {1: load_pcvec3("cb1_all", conv1_b_ext),
                2: load_pcvec3("cb2_all", conv2_b_ext)}
        fcb_all = {}
        for which, bext in ((1, fc1_b_ext), (2, fc2_b_ext)):
            t = persist.tile([128, 3 * NCC], f32, name=f"fcb{which}_all")
            nc.gpsimd.dma_start(
                out=t, in_=bext.rearrange("i (c p) -> p (i c)", p=128))
            fcb_all[which] = t

        # ------------- weight prep: gpsimd cast-DMAs + gpsimd squares ------
        # k-major order matches the conv's k-outer matmul consumption order
        W_ORDER = [(k * G + ci, ci, k) for k in range(KW) for ci in range(G)]

        def wprep_state():
            return ([None] * (KW * G), [])

        def wprep_dma(i, which, state, lo, hi):
            """bf16 weight loads on the (otherwise idle) sync HWDGE ring."""
            vext = conv1_v_ext if which == 1 else conv2_v_ext
            W, _ = state
            for idx, ci, k in W_ORDER[lo:hi]:
                W[idx] = wpool.tile(
                    [128, TCH], bf16, tag=f"w{which}_{idx}",
                    name=f"w{which}_{i}_{idx}")
                nc.sync.dma_start(
                    out=W[idx], in_=vext[i, k, ci * 128:(ci + 1) * 128, :])

        def wprep_sq(i, which, state, lo, hi):
            """bf16 squares on DVE (~0.35us each; gpsimd takes 1-2us)."""
            W, vsqs = state
            for idx, ci, k in W_ORDER[lo:hi]:
                vsq = scr.tile([128, TCH], bf16, tag="vsq", bufs=12,
                               name=f"vsq_{i}_{which}_{k}_{ci}")
                nc.vector.tensor_tensor(out=vsq, in0=W[idx], in1=W[idx],
                                        op=OP.mult)
                vsqs.append(vsq)

        # W1_0 loads at startup, behind the sync ring's x share
        st1 = wprep_state()
        wprep_dma(0, 1, st1, 0, KW * G)

        # ------------- input stats: quarter-wide ops as x arrives ----------
        # 1024-col ops halve the op count vs per-chunk; sums on DVE,
        # squares on ACT except the last quarter's (DVE) so the tail
        # finishes on two engines in parallel.
        xsum_cur = small.tile([128, G, QT], f32, tag="xsumi", name="xsum_in")
        xsq_cur = small.tile([128, G, QT], f32, tag="xsqi", name="xsq_in")
        for q in range(QT):
            for g in range(G):
                qsl = slice(q * TQ, (q + 1) * TQ)
                nc.vector.tensor_reduce(
                    xsum_cur[:, g, q:q + 1], x_cur[g][:, qsl],
                    axis=AX.X, op=OP.add)
                dst = scr.tile([128, TQ], bf16, tag="sqdw",
                               name=f"sqd_xin_{g}_{q}", bufs=3)
                if q < QT - 1:
                    nc.scalar.activation(
                        out=dst, in_=x_cur[g][:, qsl], func=AF.Square,
                        bias=zero_col, accum_out=xsq_cur[:, g, q:q + 1])
                else:
                    nc.vector.affine_mul_reduce(
                        out=dst, accum_out=xsq_cur[:, g, q:q + 1],
                        in0=x_cur[g][:, qsl], in1=x_cur[g][:, qsl],
                        scale=1.0, bias=0.0)
        # W1_0's squares run on DVE once its loads land (pre-conv)
        wprep_sq(0, 1, st1, 0, KW * G)

        # ------------- DVE rsqrt (no ACT sqrt -> no table switch) ----------
        def emit_rsqrt(dst, v, tag, iters=2):
            nc.vector.tensor_scalar(
                dst.bitcast(i32), v.bitcast(i32), 1, None,
                OP.logical_shift_right)
            nc.vector.tensor_scalar(
                dst.bitcast(i32), dst.bitcast(i32), -1, 0x5F3759DF,
                OP.mult, OP.add)
            t = small.tile(list(v.shape), f32, tag=f"nr_{tag}", name=f"nr_{tag}")
            for _ in range(iters):
                nc.vector.tensor_tensor(out=t, in0=dst, in1=dst, op=OP.mult)
                nc.vector.tensor_tensor(out=t, in0=t, in1=v, op=OP.mult)
                nc.vector.tensor_scalar(t, t, -0.5, 1.5, OP.mult, OP.add)
                nc.vector.tensor_tensor(out=dst, in0=dst, in1=t, op=OP.mult)
            return dst

        # 1/sqrt(alpha), 1/alpha, 1/g^2 for all layers (off the critical path)
        sqS_t, invA_t, rg2_t = {}, {}, {}
        for which in (1, 2):
            allt = persist.tile([128, 3 * G], f32, name=f"sqS{which}_all")
            emit_rsqrt(allt, alpha_t[which][:, 0:3 * G], f"sa{which}")
            sqS_t[which] = allt
            inv = persist.tile([128, 3 * G], f32, name=f"invA{which}_all")
            nc.vector.reciprocal(inv, alpha_t[which][:, 0:3 * G])
            invA_t[which] = inv
        rg = persist.tile([128, 3 * G], f32, name="rg1")
        nc.vector.reciprocal(rg, g_t[1][:, 0:3 * G])
        rg2 = persist.tile([128, 3 * G], f32, name="rg2_1")
        nc.vector.tensor_tensor(out=rg2, in0=rg, in1=rg, op=OP.mult)
        rg2_t[1] = rg2

        # ------------- fc + style coefficients -------------
        def fc_mm(i, which):
            fcw = fcw_tiles.pop((i, which))
            hps = psm.tile([128, NCC], f32, tag="pm", name=f"hps_{i}_{which}")
            for cc in range(NCC):
                nc.tensor.matmul(
                    hps[:, cc:cc + 1],
                    fcw[:, cc * 128:(cc + 1) * 128],
                    s_sb,
                    start=(cc == 0), stop=(cc == NCC - 1))
            h_sb = small.tile([128, NCC], f32, tag="hsb",
                              name=f"h_{i}_{which}", bufs=2)
            nc.vector.tensor_tensor(
                out=h_sb, in0=hps,
                in1=fcb_all[which][:, i * NCC:(i + 1) * NCC], op=OP.add)
            return h_sb

        def emit_coef(tag, h_sb, alpha):
            q = small.tile([128, G], f32, tag="q", name=f"q_{tag}")
            nc.vector.scalar_tensor_tensor(
                out=q, in0=h_sb[:, 0:G], scalar=1.0, in1=alpha,
                op0=OP.add, op1=OP.mult)
            ab = small.tile([128, G], f32, tag="ab", name=f"ab_{tag}")
            nc.vector.tensor_tensor(out=ab, in0=h_sb[:, G:2 * G], in1=alpha,
                                    op=OP.mult)
            return q, ab

        hcoef = {}
        for which in (1, 2):
            h = fc_mm(0, which)
            hcoef[(0, which)] = emit_coef(f"l0_{which}", h,
                                          lay(alpha_t[which], 0))

        # ------------- weight-norm, split PE / DVE phases -------------
        def wprep_norm_mm(i, which, state):
            """PE norm matmuls + transposes; ends with nsq in SBUF (via one
            DVE copy so the psm banks recycle fast). Barrier PE filler."""
            W, vsqs = state
            normps = psm.tile([1, C], f32, tag="pm", name=f"norm_{i}_{which}")
            for n, vsq in enumerate(vsqs):
                nc.tensor.matmul(
                    normps, ones_col, vsq,
                    start=(n == 0), stop=(n == KW * G - 1))
            nrow = small.tile([1, C], f32, tag="nrow", name=f"nrow_{i}_{which}",
                              bufs=2)
            nc.scalar.activation(out=nrow, in_=normps, func=AF.Copy)
            ps_t = psm.tile([128, G], f32, tag="pm", name=f"wnt_{i}_{which}")
            for g in range(G):
                nc.tensor.matmul(
                    ps_t[:, g:g + 1], nrow[0:1, g * 128:(g + 1) * 128], ident1,
                    is_transpose=True, start=(g == 0), stop=(g == G - 1))
            nsq = small.tile([128, G], f32, tag=f"nsq{which}",
                             name=f"nsq_{i}_{which}", bufs=2)
            nc.vector.tensor_scalar(nsq, ps_t, 0.0, None, OP.add)
            return W, nsq

        def wprep_epsc(i, nsq):
            """Per-channel epsilon EPS*||v||^2/g^2 for the wnS1-free stats."""
            epsc = small.tile([128, G], f32, tag="epsc", name=f"epsc_{i}",
                              bufs=2)
            nc.vector.scalar_tensor_tensor(
                out=epsc, in0=nsq, scalar=EPS, in1=lay(rg2_t[1], i),
                op0=OP.mult, op1=OP.mult)
            return epsc

        def wprep_fin(i, which, nsq):
            """g/||v|| via DVE rsqrt (conv2 only: its scale must be applied
            at eviction since the residual add isn't normalized)."""
            wnS = small.tile([128, G], f32, tag=f"wns{which}",
                             name=f"wns_{i}_{which}", bufs=2)
            emit_rsqrt(wnS, nsq, f"wn{which}")
            nc.vector.tensor_tensor(out=wnS, in0=wnS, in1=lay(g_t[which], i),
                                    op=OP.mult)
            return wnS

        # ------------- warm-bridge dummy matmuls -------------
        def warm_fill(tag, n, rhs=None):
            """n back-to-back matmuls into a scratch psm tile: keeps the PE
            HAM at K=8/8 through a stats barrier. rhs gates the start."""
            if n <= 0:
                return
            mv = rhs if rhs is not None else warm_src
            st = eps_col if mv.dtype == f32 else ones_col
            jp = psm.tile([1, TCH], f32, tag="pm", name=f"jk_{tag}")
            for m in range(n):
                nc.tensor.matmul(jp, st, mv[:, 0:TCH] if mv.shape[1] > TCH
                                 else mv, start=(m == 0), stop=(m == n - 1))

        # ------------- stats chain -------------
        def emit_stats(tag, sum3d, sq3d, coef, epsc=None):
            """sinS/sinB from per-chunk raw sums. epsc: per-channel epsilon
            tensor (wnS1-folded path) or None (EPS scalar, exact scale)."""
            q, ab = coef
            sums = small.tile([128, G], f32, tag="sums", name=f"sums_{tag}")
            nc.vector.tensor_reduce(sums, sum3d, axis=AX.X, op=OP.add)
            sqs = small.tile([128, G], f32, tag="sqs", name=f"sqs_{tag}")
            nc.vector.tensor_reduce(sqs, sq3d, axis=AX.X, op=OP.add)
            mu = small.tile([128, G], f32, tag="mu", name=f"mu_{tag}")
            nc.vector.tensor_scalar(mu, sums, 1.0 / T, None, OP.mult)
            if epsc is None:
                nc.vector.tensor_scalar(sqs, sqs, 1.0 / T, EPS, OP.mult, OP.add)
            else:
                nc.vector.tensor_scalar(sqs, sqs, 1.0 / T, None, OP.mult)
            var = small.tile([128, G], f32, tag="var", name=f"var_{tag}")
            nc.vector.tensor_tensor(out=var, in0=mu, in1=mu, op=OP.mult)
            nc.vector.tensor_tensor(out=var, in0=sqs, in1=var, op=OP.subtract)
            if epsc is not None:
                nc.vector.tensor_tensor(out=var, in0=var, in1=epsc, op=OP.add)
            istd = small.tile([128, G], f32, tag="istd", name=f"istd_{tag}")
            emit_rsqrt(istd, var, "istd", iters=1)
            sinS = small.tile([128, G], f32, tag="sinS", name=f"sinS_{tag}")
            nc.vector.tensor_tensor(out=sinS, in0=q, in1=istd, op=OP.mult)
            sinB = small.tile([128, G], f32, tag="sinB", name=f"sinB_{tag}")
            nc.vector.tensor_tensor(out=sinB, in0=mu, in1=sinS, op=OP.mult)
            nc.vector.tensor_tensor(out=sinB, in0=ab, in1=sinB, op=OP.subtract)
            return sinS, sinB

        def snake_chunk(tag, cj, src_fn, dst_fn, sinS, sinB, sqS, invA):
            """dst = (t + sin(wrap(t))^2 * sqS^2) * invA, t = sinS*x + sinB,
            for one 512-col chunk, all groups."""
            csl = slice(cj * TCH, (cj + 1) * TCH)
            for g in range(G):
                t_g = scr.tile([128, TCH], f16, tag="ang",
                               name=f"ang_{tag}_{cj}_{g}", bufs=5)
                w_g = scr.tile([128, TCH], f16, tag="wrap",
                               name=f"wrap_{tag}_{cj}_{g}", bufs=5)
                sin_g = scr.tile([128, TCH], f16, tag="sin",
                                 name=f"sin_{tag}_{cj}_{g}", bufs=5)
                nc.gpsimd.tensor_scalar(
                    t_g, src_fn(g)[:, csl],
                    sinS[:, g:g + 1], sinB[:, g:g + 1],
                    OP.mult, OP.add)
                nc.vector.add_range_wrap(w_g, t_g, 0.0, PI, 2.0 * PI)
                for _ in range(N_WRAPS - 1):
                    nc.vector.add_range_wrap(w_g, w_g, 0.0, PI, 2.0 * PI)
                nc.scalar.activation(out=sin_g, in_=w_g,
                                     func=AF.Sin, bias=zero_col)
                nc.scalar.activation(out=sin_g, in_=sin_g,
                                     func=AF.Square,
                                     scale=sqS[:, g:g + 1], bias=zero_col)
                nc.vector.scalar_tensor_tensor(
                    out=dst_fn(g)[:, csl], in0=t_g,
                    scalar=invA[:, g:g + 1], in1=sin_g,
                    op0=OP.mult, op1=OP.add)

        def emit_sq_chunk(src_ap, slot_ap, parity, tag):
            dst = scr.tile([128, TCH], bf16, tag="sqd", name=f"sqd_{tag}",
                           bufs=3)
            if parity:
                nc.scalar.activation(out=dst, in_=src_ap, func=AF.Square,
                                     bias=zero_col, accum_out=slot_ap)
            else:
                nc.vector.affine_mul_reduce(
                    out=dst, accum_out=slot_ap, in0=src_ap, in1=src_ap,
                    scale=1.0, bias=0.0)

        def emit_conv(tag, W, src_pad, pad, d, evict_fn, post_fn=None,
                      snake_fn=None, inject=None, pre_fn=None):
            """Conv waves, k-outer (only the last k-group of matmuls depends
            on snake chunk tj+1), with JIT snake production and mid-conv
            injection hooks."""
            produced = 0
            for wi, wave in enumerate(WAVES):
                if snake_fn is not None:
                    need = min(NT, wave[-1] + 2)
                    while produced < need:
                        snake_fn(produced)
                        produced += 1
                if pre_fn is not None:
                    pre_fn(wi, wave)
                for co in range(G):
                    pts = [
                        psc.tile([128, TCH], f32, tag="pc",
                                 name=f"ps_{tag}_{co}_{tj}")
                        for tj in wave
                    ]
                    for k in range(KW):
                        for ci in range(G):
                            first = (k == 0 and ci == 0)
                            last = (k == KW - 1 and ci == G - 1)
                            for pt, tj in zip(pts, wave):
                                off = pad + tj * TCH + (k - 1) * d
                                nc.tensor.matmul(
                                    pt,
                                    W[k * G + ci][:, co * 128:(co + 1) * 128],
                                    src_pad[ci][:, off:off + TCH],
                                    start=first, stop=last)
                    for pt, tj in zip(pts, wave):
                        evict_fn(co, tj, pt)
                        if post_fn is not None:
                            post_fn(co, tj, pt)
                if inject is not None and wi in inject:
                    for fn in inject.pop(wi):
                        fn()
            if snake_fn is not None:
                while produced < NT:
                    snake_fn(produced)
                    produced += 1

        # stats for the input of conv1_0 (exact scale -> scalar EPS path)
        sinS1, sinB1 = emit_stats("a1_0", xsum_cur, xsq_cur, hcoef.pop((0, 1)))
        # PE warm-up bridge: gated on the last x quarter's arrival
        warm_fill("st", 8, rhs=x_cur[G - 1][:, T - TCH:T])

        # ------------- iterations -------------
        pending_bias = None
        st2 = None

        for i in range(n_iters):
            d = DILATIONS[i]
            coef2 = hcoef.pop((i, 2))

            def snake1(cj, i=i, sS=sinS1, sB=sinB1):
                snake_chunk(f"s1_{i}", cj,
                            src_fn=lambda g: x_cur[g][:, 0:T],
                            dst_fn=lambda g: b1pad[g][:, PADL:PADL + T],
                            sinS=sS, sinB=sB,
                            sqS=lay(sqS_t[1], i), invA=lay(invA_t[1], i))

            c1sum = small.tile([128, G, NT], f32, tag="c1sum", name=f"c1sum_{i}")
            c1sq = small.tile([128, G, NT], f32, tag="c1sq", name=f"c1sq_{i}")

            # conv1 evicts RAW output: the weight-norm scale folds into the
            # following instance norm (per-channel epsilon), and conv1's bias
            # is absorbed exactly by its mean subtraction.
            def evict1(co, tj, pt, c1sum=c1sum):
                dst = cb2pad[co][:, 1 + tj * TCH: 1 + (tj + 1) * TCH]
                nc.scalar.activation(
                    out=dst, in_=pt, func=AF.Identity,
                    bias=zero_col, accum_out=c1sum[:, co, tj:tj + 1])

            def post1(co, tj, pt, i=i, c1sq=c1sq):
                src_ap = cb2pad[co][:, 1 + tj * TCH: 1 + (tj + 1) * TCH]
                dst = scr.tile([128, TCH], bf16, tag="sqd",
                               name=f"sqd_c1_{i}_{co}_{tj}", bufs=3)
                nc.vector.affine_mul_reduce(
                    out=dst, accum_out=c1sq[:, co, tj:tj + 1],
                    in0=src_ap, in1=src_ap, scale=1.0, bias=0.0)

            # conv2_i weight pipeline rides conv1_i's waves
            st2 = wprep_state()
            nsq1_hold = {}
            inject1 = {
                1: [lambda st2=st2, i=i: wprep_dma(i, 2, st2, 0, 6)],
                2: [lambda st2=st2, i=i: wprep_dma(i, 2, st2, 6, 12)],
                4: [lambda st2=st2, i=i: wprep_sq(i, 2, st2, 0, 6)],
                5: [lambda st2=st2, i=i: wprep_sq(i, 2, st2, 6, 12)],
            }
            if i == 0:
                # W1_0's norm matmuls run mid-conv (its squares ran at
                # startup) so the 12 vsq buffers recycle before W2_0's
                # squares need them.
                inject1[3] = [lambda: nsq1_hold.update(
                    n=wprep_norm_mm(0, 1, st1)[1])]
            if i < n_iters - 1:
                inject1.setdefault(6, []).append(
                    lambda i=i: (fc_dma(i + 1, 1), fc_dma(i + 1, 2)))

            def pre1(wi, wave, i=i):
                if wi == 0:
                    # phase-B warm bridge: gated on snake1 chunk 0, group 0
                    warm_fill(f"b1_{i}", 12, rhs=b1pad[0][:, PADL:PADL + TCH])

            emit_conv(f"c1_{i}", st1[0], b1pad, PADL, d, evict1, post1,
                      snake_fn=snake1, inject=inject1, pre_fn=pre1)

            # barrier c1_i -> c2_i: norm matmuls as PE filler, then bridge
            _, nsq2 = wprep_norm_mm(i, 2, st2)
            warm_fill(f"a2_{i}", 10)
            if i == 0:
                epsc1 = wprep_epsc(0, nsq1_hold["n"])
            sinS2, sinB2 = emit_stats(f"a2_{i}", c1sum, c1sq, coef2,
                                      epsc=epsc1)
            wnS2 = wprep_fin(i, 2, nsq2)

            def snake2(cj, i=i, sS=sinS2, sB=sinB2):
                snake_chunk(f"s2_{i}", cj,
                            src_fn=lambda g: cb2pad[g][:, 1:1 + T],
                            dst_fn=lambda g: cb2pad[g][:, 1:1 + T],
                            sinS=sS, sinB=sB,
                            sqS=lay(sqS_t[2], i), invA=lay(invA_t[2], i))

            # conv2 bias: accumulate; apply per chunk under the last conv2
            if pending_bias is None:
                pending_bias = small.tile([128, G], f32, tag="pend",
                                          name="pending_bias", bufs=1)
                nc.vector.tensor_copy(pending_bias, lay(cb_t[2], i))
            else:
                nc.vector.tensor_tensor(out=pending_bias, in0=pending_bias,
                                        in1=lay(cb_t[2], i), op=OP.add)

            last = (i == n_iters - 1)

            xsum_nxt = small.tile([128, G, NT], f32, tag="xsum", name=f"xsum_{i}")
            xsq_nxt = small.tile([128, G, NT], f32, tag="xsq", name=f"xsq_{i}")

            def evict2(co, tj, pt, wnS2=wnS2, xsum_nxt=xsum_nxt):
                sl = x_cur[co][:, tj * TCH:(tj + 1) * TCH]
                nc.vector.scalar_tensor_tensor(
                    out=sl, in0=pt, scalar=wnS2[:, co:co + 1], in1=sl,
                    op0=OP.mult, op1=OP.add,
                    accum_out=xsum_nxt[:, co, tj:tj + 1])

            def post2(co, tj, pt, i=i, xsq_nxt=xsq_nxt, last=last):
                sl = x_cur[co][:, tj * TCH:(tj + 1) * TCH]
                if last:
                    # sync ring only: scalar-ring issues would eat ACT time
                    nc.sync.dma_start(
                        out=out_ext[co * 128:(co + 1) * 128,
                                    tj * TCH:(tj + 1) * TCH],
                        in_=sl)
                else:
                    emit_sq_chunk(sl, xsq_nxt[:, co, tj:tj + 1],
                                  parity=1, tag=f"x_{i}_{co}_{tj}")

            # next layer's conv1 weights + fc ride conv2_i's waves
            inject2 = {}
            st1n = wprep_state()
            if i < n_iters - 1:
                def fc_next(i=i):
                    for which in (1, 2):
                        h = fc_mm(i + 1, which)
                        hcoef[(i + 1, which)] = emit_coef(
                            f"l{i + 1}_{which}", h,
                            lay(alpha_t[which], i + 1))
                inject2 = {
                    1: [lambda st1n=st1n, i=i: wprep_dma(i + 1, 1, st1n, 0, 6)],
                    2: [lambda st1n=st1n, i=i: wprep_dma(i + 1, 1, st1n, 6, 12)],
                    4: [lambda st1n=st1n, i=i: wprep_sq(i + 1, 1, st1n, 0, 6)],
                    5: [lambda st1n=st1n, i=i: wprep_sq(i + 1, 1, st1n, 6, 12)],
                    6: [fc_next],
                }

            def pre2(wi, wave, i=i, last=last, pending_bias=pending_bias):
                if wi == 0:
                    warm_fill(f"b2_{i}", 12, rhs=cb2pad[0][:, 1:1 + TCH])
                if last:
                    # deferred-bias adds on ACT (this window has no sq-accum;
                    # DVE is saturated and gpsimd f32 RMW costs 7.5us/op)
                    for co in range(G):
                        for tj in wave:
                            sl = x_cur[co][:, tj * TCH:(tj + 1) * TCH]
                            nc.scalar.activation(
                                out=sl, in_=sl, func=AF.Identity,
                                bias=pending_bias[:, co:co + 1])

            emit_conv(f"c2_{i}", st2[0], cb2pad, 1, 1, evict2, post2,
                      snake_fn=snake2, inject=inject2, pre_fn=pre2)
            xsum_cur, xsq_cur = xsum_nxt, xsq_nxt

            if i < n_iters - 1:
                # barrier c2_i -> c1_{i+1}
                _, nsq1n = wprep_norm_mm(i + 1, 1, st1n)
                warm_fill(f"a1_{i + 1}", 10)
                epsc1 = wprep_epsc(i + 1, nsq1n)
                sinS1, sinB1 = emit_stats(
                    f"a1_{i + 1}", xsum_cur, xsq_cur, hcoef.pop((i + 1, 1)))
                st1 = st1n

    return nc


def make_in_maps(inputs, T=T_FULL):
    import ml_dtypes
    npf = lambda v: np.asarray(v, dtype=np.float32)
    npb = lambda v: np.asarray(v, dtype=np.float32).astype(ml_dtypes.bfloat16)
    x = npf(inputs["x"])
    s = npf(inputs["s"])
    shared = {
        "fc1_w": npf(inputs["fc1_w"]),
        "fc1_b": npf(inputs["fc1_b"]),
        "alpha1": npf(inputs["alpha1"]).reshape(3, C),
        "conv1_v": npb(inputs["conv1_v"]),
        "conv1_g": npf(inputs["conv1_g"]),
        "conv1_b": npf(inputs["conv1_b"]),
        "fc2_w": npf(inputs["fc2_w"]),
        "fc2_b": npf(inputs["fc2_b"]),
        "alpha2": npf(inputs["alpha2"]).reshape(3, C),
        "conv2_v": npb(inputs["conv2_v"]),
        "conv2_g": npf(inputs["conv2_g"]),
        "conv2_b": npf(inputs["conv2_b"]),
    }
    in_maps = []
    for b in range(N_CORES):
        m = dict(shared)
        m["x"] = np.ascontiguousarray(x[b, :T, :].T)
        m["s"] = np.ascontiguousarray(s[b].reshape(S, 1))
        in_maps.append(m)
    return in_maps


_CACHED = {}


def kernel(**inputs) -> np.ndarray:
    from concourse.bass_utils import run_bass_kernel_spmd

    max_alpha = float(max(np.abs(np.asarray(inputs["alpha1"])).max(),
                          np.abs(np.asarray(inputs["alpha2"])).max()))
    key = ("nc", max_alpha)
    if key not in _CACHED:
        nc = build_nc(T_FULL, max_alpha=max_alpha)
        nc.finalize()
        _CACHED[key] = nc
    nc = _CACHED[key]
    in_maps = make_in_maps(inputs, T_FULL)
    res = run_bass_kernel_spmd(nc, in_maps, core_ids=list(range(N_CORES)))
    out = np.stack(
        [np.asarray(res.results[i]["out"]).T for i in range(N_CORES)], axis=0)
    return np.ascontiguousarray(out).astype(np.float32)
